# revision 9
# baseline (speedup 1.0000x reference)
"""FAME-GCN Trainium2 kernel.

Computes, for merged adjacency final_A = sum_k w_k A_k + (sum_k w_k A_k)^T:
    U1 = final_A @ (feature @ W3) + b3
    U2 = final_A2 @ (feature @ W1) + b1
    out = concat(U1, U2, axis=1)          # [5000, 32]

Distribution: node rows sharded 625/core across 8 NeuronCores. Per core,
temp = sum_k w_k A_k[rows] is formed on the tensor engine (scaled-identity
matmuls accumulating in PSUM), then both GCN directions run as S-stationary
matmuls: dir1 = temp^T @ S_own accumulated per column block; dir2 = temp @ S
via PE transposes of the merged stripes. The [16, N] dir1 partials are summed
across cores on the host (the reduce-scatter step of the row-sharded spmm);
biases and the final concat are also applied on host.
"""

import sys

if "/opt/trn_rl_repo" not in sys.path:
    sys.path.insert(0, "/opt/trn_rl_repo")

import numpy as np

import concourse.bacc as bacc
import concourse.mybir as mybir
from concourse.tile import TileContext
from concourse.bass_utils import run_bass_kernel_spmd

F32 = mybir.dt.float32
F32R = mybir.dt.float32r

N = 5000
NFEAT = 128
OUT = 16
K_A, K_AT = 3, 9
NCORES = 8
RS = N // NCORES  # 625 rows per core
STRIPE = 125  # rows per stripe (5 per core)
NSTRIPE = RS // STRIPE
CB = 512  # column block width
NCB = (N + CB - 1) // CB  # 10 blocks (last 392)

_CACHE = {}


def _c_blocks():
    return [(cb * CB, min(CB, N - cb * CB)) for cb in range(NCB)]


def _c_subs(cw):
    subs = []
    j = 0
    while j * 128 < cw:
        subs.append((j, min(128, cw - j * 128)))
        j += 1
    return subs


def build():
    nc = bacc.Bacc()

    a = nc.declare_dram_parameter("a", [K_A, RS, N], F32R, isOutput=False)
    at = nc.declare_dram_parameter("at", [K_AT, RS, N], F32R, isOutput=False)
    feat = nc.declare_dram_parameter("feat", [N, NFEAT], F32, isOutput=False)
    featow = nc.declare_dram_parameter("featow", [RS, NFEAT], F32, isOutput=False)
    w3 = nc.declare_dram_parameter("w3", [NFEAT, OUT], F32, isOutput=False)
    w1 = nc.declare_dram_parameter("w1", [NFEAT, OUT], F32, isOutput=False)
    ida = nc.declare_dram_parameter("ida", [128, K_A * 128], F32R, isOutput=False)
    idat = nc.declare_dram_parameter("idat", [128, K_AT * 128], F32R, isOutput=False)
    ident = nc.declare_dram_parameter("ident", [128, 128], F32R, isOutput=False)
    ident32 = nc.declare_dram_parameter("ident32", [128, 128], F32, isOutput=False)

    o1a = nc.declare_dram_parameter("o1a", [OUT, N], F32, isOutput=True)
    o1b = nc.declare_dram_parameter("o1b", [OUT, N], F32, isOutput=True)
    o2a = nc.declare_dram_parameter("o2a", [OUT, RS], F32, isOutput=True)
    o2b = nc.declare_dram_parameter("o2b", [OUT, RS], F32, isOutput=True)

    nchunks = (N + 127) // 128  # 40 column chunks of S (last width 8)

    with TileContext(nc) as tc:
        with (
            tc.tile_pool(name="persist", bufs=1) as pp,
            tc.tile_pool(name="raw", bufs=2) as rawp,
            tc.tile_pool(name="mrg", bufs=3) as mrgp,
            tc.tile_pool(name="ttp", bufs=10) as ttp,
            tc.tile_pool(name="pm", bufs=2, space="PSUM") as pmp,
            tc.tile_pool(name="pt", bufs=2, space="PSUM") as ptp,
            tc.tile_pool(name="pacc1", bufs=2, space="PSUM") as pacc1,
            tc.tile_pool(name="pd2", bufs=1, space="PSUM") as pd2,
        ):
            # ---------------- persistent tiles ----------------
            w3t = pp.tile([NFEAT, OUT], F32, tag="w3t")
            w1t = pp.tile([NFEAT, OUT], F32, tag="w1t")
            nc.sync.dma_start(out=w3t, in_=w3[:, :])
            nc.sync.dma_start(out=w1t, in_=w1[:, :])

            idat_a = pp.tile([128, K_A * 128], F32R, tag="ida")
            idat_b = pp.tile([128, K_AT * 128], F32R, tag="idat")
            nc.sync.dma_start(out=idat_a, in_=ida[:, :])
            nc.sync.dma_start(out=idat_b, in_=idat[:, :])
            id_t = pp.tile([128, 128], F32R, tag="ident")
            nc.sync.dma_start(out=id_t, in_=ident[:, :])
            id32_t = pp.tile([128, 128], F32, tag="ident32")
            nc.sync.dma_start(out=id32_t, in_=ident32[:, :])

            featT = pp.tile([NFEAT, N], F32, tag="featT")
            featTow = pp.tile([NFEAT, RS], F32, tag="featTow")
            # S chunk layouts: s3f/s1f on the global 128-grid (for dir2);
            # s3o/s1o on the per-core 125-grid (for dir1).
            s3f = pp.tile([128, nchunks * OUT], F32R, tag="s3f")
            s1f = pp.tile([128, nchunks * OUT], F32R, tag="s1f")
            s3o = pp.tile([STRIPE, NSTRIPE * OUT], F32R, tag="s3o")
            s1o = pp.tile([STRIPE, NSTRIPE * OUT], F32R, tag="s1o")

            o1sb_a = pp.tile([OUT, N], F32, tag="o1sb_a")
            o1sb_b = pp.tile([OUT, N], F32, tag="o1sb_b")
            acc2a = pp.tile([OUT, RS], F32, tag="acc2a")
            acc2b = pp.tile([OUT, RS], F32, tag="acc2b")
            nc.vector.memset(acc2a, 0.0)
            nc.vector.memset(acc2b, 0.0)

            # ---------------- preamble: S matrices ----------------
            # featT = feature^T via PE transposes of [wt,128] row chunks.
            for t in range(nchunks):
                r0 = t * 128
                wt = min(128, N - r0)
                ft = rawp.tile([128, NFEAT], F32, tag="ftile")
                nc.sync.dma_start(out=ft[:wt, :], in_=feat[r0 : r0 + wt, :])
                ptr = pmp.tile([128, 128], F32, tag="pm", name=f"pt32_{t}")
                nc.tensor.transpose(ptr[:, :wt], ft[:wt, :], id32_t[:wt, :wt])
                nc.vector.tensor_copy(out=featT[:, r0 : r0 + wt], in_=ptr[:, :wt])
            for u in range(NSTRIPE):
                r0 = u * STRIPE
                ft = rawp.tile([128, NFEAT], F32, tag="ftile")
                nc.sync.dma_start(out=ft[:STRIPE, :], in_=featow[r0 : r0 + STRIPE, :])
                ptr = pmp.tile([128, 128], F32, tag="pm", name=f"pt32o_{u}")
                nc.tensor.transpose(
                    ptr[:, :STRIPE], ft[:STRIPE, :], id32_t[:STRIPE, :STRIPE]
                )
                nc.vector.tensor_copy(
                    out=featTow[:, r0 : r0 + STRIPE], in_=ptr[:, :STRIPE]
                )
            # S = feature @ W, chunked; out [wt, 16] per chunk.
            for t in range(nchunks):
                r0 = t * 128
                wt = min(128, N - r0)
                for wtile, sdst in ((w3t, s3f), (w1t, s1f)):
                    ps = pmp.tile([128, OUT], F32, tag="pm", name=f"ps_s_{t}_{0 if sdst is s3f else 1}")
                    nc.tensor.matmul(
                        ps[:wt, :],
                        featT[:, r0 : r0 + wt],
                        wtile,
                        start=True,
                        stop=True,
                    )
                    nc.scalar.tensor_copy(
                        out=sdst[:wt, t * OUT : (t + 1) * OUT], in_=ps[:wt, :]
                    )
            for u in range(NSTRIPE):
                r0 = u * STRIPE
                for wtile, sdst in ((w3t, s3o), (w1t, s1o)):
                    ps = pmp.tile([128, OUT], F32, tag="pm", name=f"ps_s_{t}_{0 if sdst is s3f else 1}")
                    nc.tensor.matmul(
                        ps[:STRIPE, :],
                        featTow[:, r0 : r0 + STRIPE],
                        wtile,
                        start=True,
                        stop=True,
                    )
                    nc.scalar.tensor_copy(
                        out=sdst[:, u * OUT : (u + 1) * OUT], in_=ps[:STRIPE, :]
                    )

            # ---------------- main loop ----------------
            groups = (
                ("a", K_A, idat_a, s3o, s3f, acc2a, o1sb_a),
                ("b", K_AT, idat_b, s1o, s1f, acc2b, o1sb_b),
            )
            for cb, (c0, cw) in enumerate(_c_blocks()):
                subs = _c_subs(cw)
                # dir1 accumulators for this column block
                acc1 = {
                    g[0]: pacc1.tile(
                        [OUT, CB], F32, tag="acc1", name=f"acc1_{cb}_{g[0]}"
                    )
                    for g in groups
                }
                # transposed merged strips for this column block
                tt = {
                    (g[0], j): ttp.tile(
                        [128, 640], F32R, tag="tt", name=f"tt_{cb}_{g[0]}_{j}"
                    )
                    for g in groups
                    for (j, _) in subs
                }
                for sp in range(3):  # stripe pairs: rows 0-250, 250-500, 500-625
                    r0 = sp * 2 * STRIPE
                    ns = 2 if sp < 2 else 1
                    ta = rawp.tile(
                        [STRIPE, K_A, ns, cw], F32R, tag="ta", name=f"ta_{cb}_{sp}"
                    )
                    tat = rawp.tile(
                        [STRIPE, K_AT, ns, cw], F32R, tag="tat", name=f"tat_{cb}_{sp}"
                    )
                    for s in range(ns):
                        rs0 = r0 + s * STRIPE
                        nc.sync.dma_start(
                            out=ta[:, :, s, :],
                            in_=a[:, rs0 : rs0 + STRIPE, c0 : c0 + cw].rearrange(
                                "k r c -> r k c"
                            ),
                        )
                        nc.sync.dma_start(
                            out=tat[:, :, s, :],
                            in_=at[:, rs0 : rs0 + STRIPE, c0 : c0 + cw].rearrange(
                                "k r c -> r k c"
                            ),
                        )
                    for s in range(ns):
                        st = 2 * sp + s  # global stripe index 0..4
                        for gi, (gname, nk, idt, so, sf, acc2, o1sb) in enumerate(
                            groups
                        ):
                            raw = ta if gname == "a" else tat
                            # merge: pm = sum_k w_k * raw[k]
                            pm = pmp.tile([STRIPE, CB], F32, tag="pm")
                            for k in range(nk):
                                nc.tensor.matmul(
                                    pm[:, :cw],
                                    idt[:STRIPE, 128 * k : 128 * k + STRIPE],
                                    raw[:, k, s, :cw],
                                    start=(k == 0),
                                    stop=(k == nk - 1),
                                )
                            mrg = mrgp.tile([STRIPE, CB], F32R, tag="mrg")
                            nc.scalar.tensor_copy(out=mrg[:, :cw], in_=pm[:, :cw])
                            # dir1: acc1 += S_own[stripe]^T @ mrg
                            nc.tensor.matmul(
                                acc1[gname][:, :cw],
                                so[:, st * OUT : (st + 1) * OUT],
                                mrg[:, :cw],
                                start=(st == 0),
                                stop=(st == NSTRIPE - 1),
                            )
                            # transpose merged stripe into tt strips
                            for j, cjw in subs:
                                ptr = ptp.tile([128, 128], F32R, tag="pt")
                                nc.tensor.transpose(
                                    ptr[:cjw, :126],
                                    mrg[:, 128 * j : 128 * j + cjw],
                                    id_t[:STRIPE, :126],
                                )
                                nc.vector.tensor_copy(
                                    out=tt[(gname, j)][
                                        :cjw, st * STRIPE : (st + 1) * STRIPE
                                    ],
                                    in_=ptr[:cjw, :STRIPE],
                                )
                # dir2 for this column block: acc2 += sum_j S[c_sub]^T @ tt_j
                for gname, nk, idt, so, sf, acc2, o1sb in groups:
                    pda = pd2.tile([OUT, CB], F32, tag="pda")
                    pdb = pd2.tile([OUT, 128], F32, tag="pdb")
                    for idx, (j, cjw) in enumerate(subs):
                        tj = 4 * cb + j
                        start = idx == 0
                        stop = idx == len(subs) - 1
                        nc.tensor.matmul(
                            pda,
                            sf[:cjw, tj * OUT : (tj + 1) * OUT],
                            tt[(gname, j)][:cjw, :CB],
                            start=start,
                            stop=stop,
                        )
                        nc.tensor.matmul(
                            pdb,
                            sf[:cjw, tj * OUT : (tj + 1) * OUT],
                            tt[(gname, j)][:cjw, CB:640],
                            start=start,
                            stop=stop,
                        )
                    nc.vector.tensor_add(acc2[:, :CB], acc2[:, :CB], pda)
                    nc.vector.tensor_add(acc2[:, CB:RS], acc2[:, CB:RS], pdb[:, : RS - CB])
                    # dir1 result for this block -> SBUF
                    nc.scalar.tensor_copy(
                        out=o1sb[:, c0 : c0 + cw], in_=acc1[gname][:, :cw]
                    )

            nc.sync.dma_start(out=o1a[:, :], in_=o1sb_a)
            nc.sync.dma_start(out=o1b[:, :], in_=o1sb_b)
            nc.sync.dma_start(out=o2a[:, :], in_=acc2a)
            nc.sync.dma_start(out=o2b[:, :], in_=acc2b)

    nc.compile()
    return nc


def kernel(feature, A, A_t, weight_b2, weight_b, W3, b3, W1, b1, **kw):
    feature = np.asarray(feature, dtype=np.float32)
    A = np.asarray(A, dtype=np.float32)
    A_t = np.asarray(A_t, dtype=np.float32)
    w2 = np.asarray(weight_b2, dtype=np.float32).reshape(K_A)
    wb = np.asarray(weight_b, dtype=np.float32).reshape(K_AT)
    W3 = np.asarray(W3, dtype=np.float32)
    W1 = np.asarray(W1, dtype=np.float32)
    b3 = np.asarray(b3, dtype=np.float32)
    b1 = np.asarray(b1, dtype=np.float32)

    if "nc" not in _CACHE:
        _CACHE["nc"] = build()
    nc = _CACHE["nc"]

    eye = np.eye(128, dtype=np.float32)
    ida = np.concatenate([w * eye for w in w2], axis=1)
    idat = np.concatenate([w * eye for w in wb], axis=1)

    in_maps = []
    for p in range(NCORES):
        r0 = p * RS
        in_maps.append(
            {
                "a": np.ascontiguousarray(A[:, r0 : r0 + RS, :]),
                "at": np.ascontiguousarray(A_t[:, r0 : r0 + RS, :]),
                "feat": feature,
                "featow": np.ascontiguousarray(feature[r0 : r0 + RS, :]),
                "w3": W3,
                "w1": W1,
                "ida": ida,
                "idat": idat,
                "ident": eye,
                "ident32": eye,
            }
        )

    res = run_bass_kernel_spmd(nc, in_maps, core_ids=list(range(NCORES)))

    col_a = np.zeros((OUT, N), dtype=np.float32)
    col_b = np.zeros((OUT, N), dtype=np.float32)
    row_a = np.empty((OUT, N), dtype=np.float32)
    row_b = np.empty((OUT, N), dtype=np.float32)
    for p in range(NCORES):
        r = res.results[p]
        col_a += r["o1a"]
        col_b += r["o1b"]
        row_a[:, p * RS : (p + 1) * RS] = r["o2a"]
        row_b[:, p * RS : (p + 1) * RS] = r["o2b"]

    U1 = (col_a + row_a).T + b3
    U2 = (col_b + row_b).T + b1
    return np.concatenate([U1, U2], axis=1).astype(np.float32)


# revision 10
# speedup vs baseline: 1.3111x; 1.3111x over previous
"""FAME-GCN Trainium2 kernel.

Computes, for merged adjacency final_A = sum_k w_k A_k + (sum_k w_k A_k)^T:
    U1 = final_A @ (feature @ W3) + b3
    U2 = final_A2 @ (feature @ W1) + b1
    out = concat(U1, U2, axis=1)          # [5000, 32]

Distribution: node rows sharded 625/core across 8 NeuronCores. Per core,
temp = sum_k w_k A_k[rows] is formed on the tensor engine (scaled-identity
matmuls accumulating in PSUM), then both GCN directions run as S-stationary
matmuls: dir1 = temp^T @ S_own accumulated per column block; dir2 = temp @ S
via PE transposes of the merged stripes. The [16, N] dir1 partials are summed
across cores on the host (the reduce-scatter step of the row-sharded spmm);
biases and the final concat are also applied on host.
"""

import sys

if "/opt/trn_rl_repo" not in sys.path:
    sys.path.insert(0, "/opt/trn_rl_repo")

import numpy as np

import concourse.bacc as bacc
import concourse.mybir as mybir
from concourse.tile import TileContext
from concourse.bass_utils import run_bass_kernel_spmd

F32 = mybir.dt.float32
F32R = mybir.dt.float32r

N = 5000
NFEAT = 128
OUT = 16
K_A, K_AT = 3, 9
NCORES = 8
RS = N // NCORES  # 625 rows per core
STRIPE = 125  # rows per stripe (5 per core)
NSTRIPE = RS // STRIPE
CB = 512  # column block width
NCB = (N + CB - 1) // CB  # 10 blocks (last 392)

_CACHE = {}


def _c_blocks():
    return [(cb * CB, min(CB, N - cb * CB)) for cb in range(NCB)]


def _c_subs(cw):
    subs = []
    j = 0
    while j * 128 < cw:
        subs.append((j, min(128, cw - j * 128)))
        j += 1
    return subs


def build():
    nc = bacc.Bacc()

    a = nc.declare_dram_parameter("a", [K_A, RS, N], F32R, isOutput=False)
    at = nc.declare_dram_parameter("at", [K_AT, RS, N], F32R, isOutput=False)
    feat = nc.declare_dram_parameter("feat", [N, NFEAT], F32, isOutput=False)
    featow = nc.declare_dram_parameter("featow", [RS, NFEAT], F32, isOutput=False)
    w3 = nc.declare_dram_parameter("w3", [NFEAT, OUT], F32, isOutput=False)
    w1 = nc.declare_dram_parameter("w1", [NFEAT, OUT], F32, isOutput=False)
    ida = nc.declare_dram_parameter("ida", [128, K_A * 128], F32R, isOutput=False)
    idat = nc.declare_dram_parameter("idat", [128, K_AT * 128], F32R, isOutput=False)
    ident = nc.declare_dram_parameter("ident", [128, 128], F32R, isOutput=False)
    ident32 = nc.declare_dram_parameter("ident32", [128, 128], F32, isOutput=False)

    o1a = nc.declare_dram_parameter("o1a", [OUT, N], F32, isOutput=True)
    o1b = nc.declare_dram_parameter("o1b", [OUT, N], F32, isOutput=True)
    o2a = nc.declare_dram_parameter("o2a", [OUT, RS], F32, isOutput=True)
    o2b = nc.declare_dram_parameter("o2b", [OUT, RS], F32, isOutput=True)

    nchunks = (N + 127) // 128  # 40 column chunks of S (last width 8)

    with TileContext(nc) as tc:
        with (
            tc.tile_pool(name="persist", bufs=1) as pp,
            tc.tile_pool(name="raw", bufs=2) as rawp,
            tc.tile_pool(name="mrg", bufs=3) as mrgp,
            tc.tile_pool(name="ttp", bufs=10) as ttp,
            tc.tile_pool(name="pm", bufs=2, space="PSUM") as pmp,
            tc.tile_pool(name="pt", bufs=2, space="PSUM") as ptp,
            tc.tile_pool(name="pacc1", bufs=2, space="PSUM") as pacc1,
            tc.tile_pool(name="pd2", bufs=1, space="PSUM") as pd2,
        ):
            # ---------------- persistent tiles ----------------
            w3t = pp.tile([NFEAT, OUT], F32, tag="w3t")
            w1t = pp.tile([NFEAT, OUT], F32, tag="w1t")
            nc.sync.dma_start(out=w3t, in_=w3[:, :])
            nc.sync.dma_start(out=w1t, in_=w1[:, :])

            idat_a = pp.tile([128, K_A * 128], F32R, tag="ida")
            idat_b = pp.tile([128, K_AT * 128], F32R, tag="idat")
            nc.sync.dma_start(out=idat_a, in_=ida[:, :])
            nc.sync.dma_start(out=idat_b, in_=idat[:, :])
            id_t = pp.tile([128, 128], F32R, tag="ident")
            nc.sync.dma_start(out=id_t, in_=ident[:, :])
            id32_t = pp.tile([128, 128], F32, tag="ident32")
            nc.sync.dma_start(out=id32_t, in_=ident32[:, :])

            featT = pp.tile([NFEAT, N], F32, tag="featT")
            featTow = pp.tile([NFEAT, RS], F32, tag="featTow")
            # S chunk layouts: s3f/s1f on the global 128-grid (for dir2);
            # s3o/s1o on the per-core 125-grid (for dir1).
            s3f = pp.tile([128, nchunks * OUT], F32R, tag="s3f")
            s1f = pp.tile([128, nchunks * OUT], F32R, tag="s1f")
            s3o = pp.tile([STRIPE, NSTRIPE * OUT], F32R, tag="s3o")
            s1o = pp.tile([STRIPE, NSTRIPE * OUT], F32R, tag="s1o")

            o1sb_a = pp.tile([OUT, N], F32, tag="o1sb_a")
            o1sb_b = pp.tile([OUT, N], F32, tag="o1sb_b")
            acc2a = pp.tile([OUT, RS], F32, tag="acc2a")
            acc2b = pp.tile([OUT, RS], F32, tag="acc2b")
            nc.vector.memset(acc2a, 0.0)
            nc.vector.memset(acc2b, 0.0)

            # ---------------- preamble: S matrices ----------------
            # featT = feature^T via PE transposes of [wt,128] row chunks.
            for t in range(nchunks):
                r0 = t * 128
                wt = min(128, N - r0)
                ft = rawp.tile([128, NFEAT], F32, tag="ftile")
                nc.sync.dma_start(out=ft[:wt, :], in_=feat[r0 : r0 + wt, :])
                ptr = pmp.tile([128, 128], F32, tag="pm", name=f"pt32_{t}")
                nc.tensor.transpose(ptr[:, :wt], ft[:wt, :], id32_t[:wt, :wt])
                nc.vector.tensor_copy(out=featT[:, r0 : r0 + wt], in_=ptr[:, :wt])
            for u in range(NSTRIPE):
                r0 = u * STRIPE
                ft = rawp.tile([128, NFEAT], F32, tag="ftile")
                nc.sync.dma_start(out=ft[:STRIPE, :], in_=featow[r0 : r0 + STRIPE, :])
                ptr = pmp.tile([128, 128], F32, tag="pm", name=f"pt32o_{u}")
                nc.tensor.transpose(
                    ptr[:, :STRIPE], ft[:STRIPE, :], id32_t[:STRIPE, :STRIPE]
                )
                nc.vector.tensor_copy(
                    out=featTow[:, r0 : r0 + STRIPE], in_=ptr[:, :STRIPE]
                )
            # S = feature @ W, chunked; out [wt, 16] per chunk.
            for t in range(nchunks):
                r0 = t * 128
                wt = min(128, N - r0)
                for wtile, sdst in ((w3t, s3f), (w1t, s1f)):
                    ps = pmp.tile([128, OUT], F32, tag="pm", name=f"ps_s_{t}_{0 if sdst is s3f else 1}")
                    nc.tensor.matmul(
                        ps[:wt, :],
                        featT[:, r0 : r0 + wt],
                        wtile,
                        start=True,
                        stop=True,
                    )
                    nc.scalar.tensor_copy(
                        out=sdst[:wt, t * OUT : (t + 1) * OUT], in_=ps[:wt, :]
                    )
            for u in range(NSTRIPE):
                r0 = u * STRIPE
                for wtile, sdst in ((w3t, s3o), (w1t, s1o)):
                    ps = pmp.tile([128, OUT], F32, tag="pm", name=f"ps_s_{t}_{0 if sdst is s3f else 1}")
                    nc.tensor.matmul(
                        ps[:STRIPE, :],
                        featTow[:, r0 : r0 + STRIPE],
                        wtile,
                        start=True,
                        stop=True,
                    )
                    nc.scalar.tensor_copy(
                        out=sdst[:, u * OUT : (u + 1) * OUT], in_=ps[:STRIPE, :]
                    )

            # ---------------- main loop ----------------
            groups = (
                ("a", K_A, idat_a, s3o, s3f, acc2a, o1sb_a),
                ("b", K_AT, idat_b, s1o, s1f, acc2b, o1sb_b),
            )
            for cb, (c0, cw) in enumerate(_c_blocks()):
                subs = _c_subs(cw)
                # dir1 accumulators for this column block
                acc1 = {
                    g[0]: pacc1.tile(
                        [OUT, CB], F32, tag="acc1", name=f"acc1_{cb}_{g[0]}"
                    )
                    for g in groups
                }
                # transposed merged strips for this column block
                tt = {
                    (g[0], j): ttp.tile(
                        [128, 640], F32R, tag="tt", name=f"tt_{cb}_{g[0]}_{j}"
                    )
                    for g in groups
                    for (j, _) in subs
                }
                for sp in range(3):  # stripe pairs: rows 0-250, 250-500, 500-625
                    r0 = sp * 2 * STRIPE
                    ns = 2 if sp < 2 else 1
                    ta = rawp.tile(
                        [STRIPE, K_A, ns, cw], F32R, tag="ta", name=f"ta_{cb}_{sp}"
                    )
                    tat = rawp.tile(
                        [STRIPE, K_AT, ns, cw], F32R, tag="tat", name=f"tat_{cb}_{sp}"
                    )
                    for s in range(ns):
                        rs0 = r0 + s * STRIPE
                        nc.gpsimd.dma_start(
                            out=ta[:, :, s, :],
                            in_=a[:, rs0 : rs0 + STRIPE, c0 : c0 + cw].rearrange(
                                "k r c -> r k c"
                            ),
                        )
                        nc.gpsimd.dma_start(
                            out=tat[:, :, s, :],
                            in_=at[:, rs0 : rs0 + STRIPE, c0 : c0 + cw].rearrange(
                                "k r c -> r k c"
                            ),
                        )
                    for s in range(ns):
                        st = 2 * sp + s  # global stripe index 0..4
                        for gi, (gname, nk, idt, so, sf, acc2, o1sb) in enumerate(
                            groups
                        ):
                            raw = ta if gname == "a" else tat
                            # merge: pm = sum_k w_k * raw[k]
                            pm = pmp.tile([STRIPE, CB], F32, tag="pm")
                            for k in range(nk):
                                nc.tensor.matmul(
                                    pm[:, :cw],
                                    idt[:STRIPE, 128 * k : 128 * k + STRIPE],
                                    raw[:, k, s, :cw],
                                    start=(k == 0),
                                    stop=(k == nk - 1),
                                )
                            mrg = mrgp.tile([STRIPE, CB], F32R, tag="mrg")
                            nc.scalar.tensor_copy(out=mrg[:, :cw], in_=pm[:, :cw])
                            # dir1: acc1 += S_own[stripe]^T @ mrg
                            nc.tensor.matmul(
                                acc1[gname][:, :cw],
                                so[:, st * OUT : (st + 1) * OUT],
                                mrg[:, :cw],
                                start=(st == 0),
                                stop=(st == NSTRIPE - 1),
                            )
                            # transpose merged stripe into tt strips
                            for j, cjw in subs:
                                ptr = ptp.tile([128, 128], F32R, tag="pt")
                                nc.tensor.transpose(
                                    ptr[:cjw, :126],
                                    mrg[:, 128 * j : 128 * j + cjw],
                                    id_t[:STRIPE, :126],
                                )
                                nc.vector.tensor_copy(
                                    out=tt[(gname, j)][
                                        :cjw, st * STRIPE : (st + 1) * STRIPE
                                    ],
                                    in_=ptr[:cjw, :STRIPE],
                                )
                # dir2 for this column block: acc2 += sum_j S[c_sub]^T @ tt_j
                for gname, nk, idt, so, sf, acc2, o1sb in groups:
                    pda = pd2.tile([OUT, CB], F32, tag="pda")
                    pdb = pd2.tile([OUT, 128], F32, tag="pdb")
                    for idx, (j, cjw) in enumerate(subs):
                        tj = 4 * cb + j
                        start = idx == 0
                        stop = idx == len(subs) - 1
                        nc.tensor.matmul(
                            pda,
                            sf[:cjw, tj * OUT : (tj + 1) * OUT],
                            tt[(gname, j)][:cjw, :CB],
                            start=start,
                            stop=stop,
                        )
                        nc.tensor.matmul(
                            pdb,
                            sf[:cjw, tj * OUT : (tj + 1) * OUT],
                            tt[(gname, j)][:cjw, CB:640],
                            start=start,
                            stop=stop,
                        )
                    nc.vector.tensor_add(acc2[:, :CB], acc2[:, :CB], pda)
                    nc.vector.tensor_add(acc2[:, CB:RS], acc2[:, CB:RS], pdb[:, : RS - CB])
                    # dir1 result for this block -> SBUF
                    nc.scalar.tensor_copy(
                        out=o1sb[:, c0 : c0 + cw], in_=acc1[gname][:, :cw]
                    )

            nc.sync.dma_start(out=o1a[:, :], in_=o1sb_a)
            nc.sync.dma_start(out=o1b[:, :], in_=o1sb_b)
            nc.sync.dma_start(out=o2a[:, :], in_=acc2a)
            nc.sync.dma_start(out=o2b[:, :], in_=acc2b)

    nc.compile()
    return nc


def kernel(feature, A, A_t, weight_b2, weight_b, W3, b3, W1, b1, **kw):
    feature = np.asarray(feature, dtype=np.float32)
    A = np.asarray(A, dtype=np.float32)
    A_t = np.asarray(A_t, dtype=np.float32)
    w2 = np.asarray(weight_b2, dtype=np.float32).reshape(K_A)
    wb = np.asarray(weight_b, dtype=np.float32).reshape(K_AT)
    W3 = np.asarray(W3, dtype=np.float32)
    W1 = np.asarray(W1, dtype=np.float32)
    b3 = np.asarray(b3, dtype=np.float32)
    b1 = np.asarray(b1, dtype=np.float32)

    if "nc" not in _CACHE:
        _CACHE["nc"] = build()
    nc = _CACHE["nc"]

    eye = np.eye(128, dtype=np.float32)
    ida = np.concatenate([w * eye for w in w2], axis=1)
    idat = np.concatenate([w * eye for w in wb], axis=1)

    in_maps = []
    for p in range(NCORES):
        r0 = p * RS
        in_maps.append(
            {
                "a": np.ascontiguousarray(A[:, r0 : r0 + RS, :]),
                "at": np.ascontiguousarray(A_t[:, r0 : r0 + RS, :]),
                "feat": feature,
                "featow": np.ascontiguousarray(feature[r0 : r0 + RS, :]),
                "w3": W3,
                "w1": W1,
                "ida": ida,
                "idat": idat,
                "ident": eye,
                "ident32": eye,
            }
        )

    res = run_bass_kernel_spmd(nc, in_maps, core_ids=list(range(NCORES)))

    col_a = np.zeros((OUT, N), dtype=np.float32)
    col_b = np.zeros((OUT, N), dtype=np.float32)
    row_a = np.empty((OUT, N), dtype=np.float32)
    row_b = np.empty((OUT, N), dtype=np.float32)
    for p in range(NCORES):
        r = res.results[p]
        col_a += r["o1a"]
        col_b += r["o1b"]
        row_a[:, p * RS : (p + 1) * RS] = r["o2a"]
        row_b[:, p * RS : (p + 1) * RS] = r["o2b"]

    U1 = (col_a + row_a).T + b3
    U2 = (col_b + row_b).T + b1
    return np.concatenate([U1, U2], axis=1).astype(np.float32)
